# revision 1
# baseline (speedup 1.0000x reference)
"""Trainium2 Bass kernel for nn_BarrierPolicy (CBF-QP safety filter).

Data-parallel over batch: 8 cores x 32768 samples.
Phase A (per 2048-sample tile): load x in "xview" layout, PE-transpose to
"SP2" (stacked pack-2) layout, run the 3-layer MLP + dynamics matmuls on the
tensor engine, transpose results back to xview.
Phase B (full core): Kiwiel variable-fixing active-set solve of the
per-sample box-QP dual (5 iterations + closed-form finish), then
u = clip(-p + lam*g).

Layouts (per tile of 2048 samples):
  xview: SBUF (128, 128): partition r, col 16b+8s0+j <-> sample 256b+2r+s0, coord j
  SP2  : transpose of xview: partition 16b+8s0+j, col r
  padded-pair psum (for 16-row matmul outs, 32-align rule): chunk b=2q+h at
  partitions [32q,32q+16), free-slot h.
  slot : per-sample scalars (128, 16): partition r, col 2b+s0
"""
import numpy as np

B_FULL, N = 262144, 8
NCORES = 8
S = B_FULL // NCORES          # 32768 samples per core
TILE = 2048
NT = S // TILE                # 16 tiles
NSLOT = S // 128              # 256 slot cols per core
T_KIWIEL = 5
LAMCAP = float(2.0 ** 40)
EPS = 1e-12

_CACHE = {}

_CSHAPES = dict(TL2=(128, 128), TL3px=(64, 16), TL3a=(128, 2),
                TDA=(128, 128), TDG=(128, 128), ID128=(128, 128),
                B1v=(128, 1), B2v=(128, 1), B31e=(128, 1), B32e=(128, 1),
                **{f"TL1E{b}": (128, 128) for b in range(8)})


def _consts(W1, b1, W21, b21, W22, b22, W31, b31, W32, b32, A, G):
    f32 = np.float32
    out = {}
    for b in range(8):
        T = np.zeros((128, 128), f32)
        for s0 in range(2):
            T[16 * b + 8 * s0:16 * b + 8 * s0 + 8, 64 * s0:64 * s0 + 64] = W1
        out[f"TL1E{b}"] = T
    TL2 = np.zeros((128, 128), f32)
    for s0 in range(2):
        TL2[64 * s0:64 * s0 + 64, 32 * s0:32 * s0 + 32] = W21
        TL2[64 * s0:64 * s0 + 64, 64 + 32 * s0:64 + 32 * s0 + 32] = W22
    TL3px = np.zeros((64, 16), f32)
    for s0 in range(2):
        TL3px[32 * s0:32 * s0 + 32, 8 * s0:8 * s0 + 8] = W31
    TL3a = np.zeros((128, 2), f32)          # used as slice [64:128)
    for s0 in range(2):
        TL3a[64 + 32 * s0:64 + 32 * s0 + 32, s0:s0 + 1] = W32
    TDA = np.kron(np.eye(16, dtype=f32), A.T.astype(f32))         # out = A x
    TDG = np.kron(np.eye(16, dtype=f32), (-2.0 * G).astype(f32))  # out = -2 G^T x
    ID128 = np.eye(128, dtype=f32)
    B1v = np.concatenate([b1, b1]).reshape(128, 1).astype(f32)
    B2v = np.concatenate([b21, b21, b22, b22]).reshape(128, 1).astype(f32)
    B31e = np.zeros((128, 1), f32)          # bias for padded px evac (3 bases)
    for m in range(3):
        for s0 in range(2):
            B31e[32 * m + 8 * s0:32 * m + 8 * s0 + 8, 0] = b31
    B32e = np.full((128, 1), float(b32[0]), f32)
    out.update(TL2=TL2, TL3px=TL3px, TL3a=TL3a, TDA=TDA, TDG=TDG, ID128=ID128,
               B1v=B1v, B2v=B2v, B31e=B31e, B32e=B32e)
    return out


def build_kernel(nc, tc, x_d, u_d, cds):
    from concourse import mybir
    f32 = mybir.dt.float32
    AL = mybir.AluOpType
    AF = mybir.ActivationFunctionType
    XL = mybir.AxisListType.X

    with (
        tc.tile_pool(name="const", bufs=1) as cpool,
        tc.tile_pool(name="pers", bufs=1) as pers,
        tc.tile_pool(name="work", bufs=2) as work,
        tc.tile_pool(name="psA", bufs=1, space="PSUM") as psA,
        tc.tile_pool(name="psB", bufs=1, space="PSUM") as psB,
    ):
        C = {k: cpool.tile(list(v), f32, tag=k, name=k) for k, v in _CSHAPES.items()}
        for k in _CSHAPES:
            nc.sync.dma_start(C[k][:], cds[k][:])

        FC = S // 16   # 2048 xview cols per core
        def fc_tile(tag):
            return pers.tile([128, FC], f32, tag=tag, name=tag)
        x_xv, p_xv, g_xv = fc_tile("x_xv"), fc_tile("p_xv"), fc_tile("g_xv")
        gt_xv, pt_xv, q_xv = fc_tile("gt_xv"), fc_tile("pt_xv"), fc_tile("q_xv")
        zt_xv, mm_xv = fc_tile("zt_xv"), fc_tile("mm_xv")
        sc1, sc2 = fc_tile("sc1"), fc_tile("sc2")
        def sl_tile(tag):
            return pers.tile([128, NSLOT], f32, tag=tag, name=tag)
        alpha4, lfhx, sxx = sl_tile("alpha4"), sl_tile("lfhx"), sl_tile("sxx")
        c0s, viol, infs = sl_tile("c0s"), sl_tile("viol"), sl_tile("infs")
        nums, dens, lams = sl_tile("nums"), sl_tile("dens"), sl_tile("lams")
        t1s, t2s, nus, bvs = sl_tile("t1s"), sl_tile("t2s"), sl_tile("nus"), sl_tile("bvs")

        # ---------------- Phase A ----------------
        for t in range(NT):
            cs = slice(128 * t, 128 * t + 128)
            ss = slice(16 * t, 16 * t + 16)
            nc.sync.dma_start(
                x_xv[:, cs].rearrange("p (b s j) -> p b s j", b=8, s=2, j=8),
                x_d[t * TILE:(t + 1) * TILE, :].rearrange(
                    "(b r s) j -> r b s j", b=8, r=128, s=2))
            TP = psA.tile([128, 3, 128], f32, tag="TP", name="TP")
            nc.tensor.transpose(TP[:, 0, :], x_xv[:, cs], C["ID128"][:])
            xsp2 = work.tile([128, 128], f32, tag="xsp2", name="xsp2")
            nc.vector.tensor_copy(xsp2[:], TP[:, 0, :])

            h1P = psA.tile([128, 4, 128], f32, tag="h1P", name="h1P")
            x2P = psA.tile([128, 4, 128], f32, tag="x2P", name="x2P")
            LPx = psA.tile([128, 3, 128], f32, tag="LPx", name="LPx")
            alP = psA.tile([128, 3, 128], f32, tag="alP", name="alP")
            h1 = work.tile([128, 8, 128], f32, tag="h1", name="h1")
            x2 = work.tile([128, 8, 128], f32, tag="x2", name="x2")
            pxe = work.tile([128, 3, 128], f32, tag="pxe", name="pxe")
            asle = work.tile([128, 3, 128], f32, tag="asle", name="asle")

            for half in range(2):
                for bi in range(4):
                    b = 4 * half + bi
                    nc.tensor.matmul(h1P[:, bi, :], C[f"TL1E{b}"][:], xsp2[:])
                for bi in range(4):
                    b = 4 * half + bi
                    nc.scalar.activation(h1[:, b, :], h1P[:, bi, :], AF.Relu,
                                         bias=C["B1v"][:])
                for bi in range(4):
                    b = 4 * half + bi
                    nc.tensor.matmul(x2P[:, bi, :], C["TL2"][:], h1[:, b, :])
                for bi in range(4):
                    b = 4 * half + bi
                    nc.scalar.activation(x2[:, b, :], x2P[:, bi, :], AF.Relu,
                                         bias=C["B2v"][:])
                for bi in range(4):
                    b = 4 * half + bi
                    m3, k3 = b % 3, b // 3
                    nc.tensor.matmul(LPx[32 * m3:32 * m3 + 16, k3, :],
                                     C["TL3px"][:], x2[0:64, b, :])
                    nc.tensor.matmul(alP[32 * m3:32 * m3 + 2, k3, :],
                                     C["TL3a"][64:128, :], x2[64:128, b, :])
            nc.gpsimd.memset(pxe[:], 0.0)
            nc.gpsimd.memset(asle[:], 0.0)
            for m in range(3):
                kk = 3 if m < 2 else 2
                nc.vector.tensor_scalar(pxe[32 * m:32 * m + 16, 0:kk, :],
                                        LPx[32 * m:32 * m + 16, 0:kk, :],
                                        C["B31e"][32 * m:32 * m + 16, :], None,
                                        AL.add)
                nc.scalar.activation(asle[32 * m:32 * m + 2, 0:kk, :],
                                     alP[32 * m:32 * m + 2, 0:kk, :], AF.Sigmoid,
                                     bias=C["B32e"][32 * m:32 * m + 2, :])

            nc.tensor.matmul(TP[:, 1, :], C["TDA"][:], xsp2[:])
            nc.tensor.matmul(TP[:, 2, :], C["TDG"][:], xsp2[:])
            axs = work.tile([128, 128], f32, tag="axs", name="axs")
            gsp2 = work.tile([128, 128], f32, tag="gsp2", name="gsp2")
            nc.vector.tensor_copy(axs[:], TP[:, 1, :])
            nc.scalar.activation(gsp2[:], TP[:, 2, :], AF.Copy)

            # transposes back to xview
            trP = psB.tile([128, 2, 128], f32, tag="trP", name="trP")
            nc.tensor.transpose(trP[:, 0, :], gsp2[:], C["ID128"][:])
            nc.tensor.transpose(trP[:, 1, :], axs[:], C["ID128"][:])
            nc.scalar.activation(g_xv[:, cs], trP[:, 0, :], AF.Copy)
            prodA = work.tile([128, 128], f32, tag="prodA", name="prodA")
            nc.vector.scalar_tensor_tensor(prodA[:], trP[:, 1, :], -2.0,
                                           x_xv[:, cs], AL.mult, AL.mult)
            nc.vector.tensor_reduce(lfhx[:, ss],
                                    prodA[:].rearrange("p (c j) -> p c j", j=8),
                                    XL, AL.add)
            sqx = work.tile([128, 128], f32, tag="sqx", name="sqx")
            nc.scalar.activation(sqx[:], x_xv[:, cs], AF.Square)
            nc.vector.tensor_reduce(sxx[:, ss],
                                    sqx[:].rearrange("p (c j) -> p c j", j=8),
                                    XL, AL.add)

            pxtP = psB.tile([128, 3, 128], f32, tag="pxtP", name="pxtP")
            altP = psB.tile([128, 3, 128], f32, tag="altP", name="altP")
            for k in range(3):
                nc.tensor.transpose(pxtP[:, k, :], pxe[:, k, :], C["ID128"][:])
                nc.tensor.transpose(altP[:, k, :], asle[:, k, :], C["ID128"][:])
            for k in range(3):
                nm = 3 if k < 2 else 2
                dstp = p_xv[:, cs].rearrange("p (b s j) -> p b s j",
                                             b=8, s=2, j=8)[:, 3 * k:3 * k + nm, :, :]
                srcp = pxtP[:, k, :].rearrange("p (m g s j) -> p m g s j",
                                               m=4, g=2, s=2, j=8)[:, 0:nm, 0, :, :]
                nc.vector.tensor_copy(dstp, srcp)
                dsta = alpha4[:, ss].rearrange("p (b s) -> p b s",
                                               b=8, s=2)[:, 3 * k:3 * k + nm, :]
                srca = altP[:, k, :].rearrange("p (m g) -> p m g",
                                               m=4, g=32)[:, 0:nm, 0:2]
                nc.vector.tensor_copy(dsta, srca)

        # ---------------- Phase B ----------------
        x3 = lambda ap: ap.rearrange("p (c j) -> p c j", j=8)
        bc = lambda ap: ap.broadcast_to((128, NSLOT, 8))
        V, GP, SC = nc.vector, nc.gpsimd, nc.scalar

        GP.tensor_scalar(alpha4[:], alpha4[:], 4.0, None, AL.mult)
        GP.tensor_scalar(t1s[:], sxx[:], -1.0, 16.0, AL.mult, AL.add)
        V.tensor_tensor(t2s[:], alpha4[:], t1s[:], AL.mult)
        V.tensor_tensor(c0s[:], t2s[:], lfhx[:], AL.add)

        SC.sign(sc1[:], g_xv[:])                                  # sigma
        V.tensor_tensor(pt_xv[:], sc1[:], p_xv[:], AL.mult)       # pt
        GP.tensor_scalar(zt_xv[:], pt_xv[:], -1.0, None, AL.mult)  # zt0
        SC.activation(gt_xv[:], g_xv[:], AF.Abs)
        SC.activation(q_xv[:], g_xv[:], AF.Square)
        V.memset(mm_xv[:], 1.0)

        V.tensor_scalar(sc2[:], p_xv[:], -1.0, 1.0, AL.mult, AL.min)
        V.tensor_scalar(sc2[:], sc2[:], -1.0, None, AL.max)
        V.tensor_tensor(sc2[:], g_xv[:], sc2[:], AL.mult)
        V.tensor_reduce(t1s[:], x3(sc2[:]), XL, AL.add)
        V.tensor_tensor(t1s[:], c0s[:], t1s[:], AL.add)
        GP.tensor_scalar(viol[:], t1s[:], 0.0, None, AL.is_lt)
        V.tensor_reduce(t2s[:], x3(gt_xv[:]), XL, AL.add)
        V.tensor_tensor(t2s[:], c0s[:], t2s[:], AL.add)
        GP.tensor_scalar(infs[:], t2s[:], 0.0, None, AL.is_lt)
        V.tensor_tensor(infs[:], infs[:], viol[:], AL.mult)

        def calc_num_den():
            V.tensor_tensor(sc1[:], gt_xv[:], zt_xv[:], AL.mult)
            V.tensor_reduce(nums[:], x3(sc1[:]), XL, AL.add)
            V.tensor_tensor(nums[:], c0s[:], nums[:], AL.add)
            GP.tensor_tensor(sc2[:], q_xv[:], mm_xv[:], AL.mult)
            V.tensor_reduce(dens[:], x3(sc2[:]), XL, AL.add)

        def calc_lam():
            GP.tensor_scalar(t1s[:], dens[:], EPS, None, AL.add)
            V.reciprocal(t2s[:], t1s[:])
            V.scalar_tensor_tensor(lams[:], nums[:], -1.0, t2s[:], AL.mult, AL.mult)
            V.tensor_tensor(lams[:], lams[:], viol[:], AL.mult)

        calc_num_den()
        for _ in range(T_KIWIEL):
            calc_lam()
            V.tensor_tensor(x3(sc1[:]), bc(lams[:]), x3(gt_xv[:]), AL.mult)
            V.tensor_tensor(sc1[:], sc1[:], pt_xv[:], AL.subtract)   # ur
            V.tensor_scalar(sc2[:], sc1[:], 1.0, -1.0, AL.min, AL.max)
            V.tensor_tensor(sc2[:], gt_xv[:], sc2[:], AL.mult)
            V.tensor_reduce(t1s[:], x3(sc2[:]), XL, AL.add)
            V.tensor_tensor(t1s[:], c0s[:], t1s[:], AL.add)          # c
            GP.tensor_scalar(nus[:], t1s[:], 0.0, None, AL.is_lt)    # needup
            GP.tensor_scalar(bvs[:], nus[:], 2.0, -1.0, AL.mult, AL.add)
            # fix = M * 1{B*ur >= 1}  (== M*(NU*m1 + (1-NU)*m2))
            V.tensor_tensor(x3(sc2[:]), bc(bvs[:]), x3(sc1[:]), AL.mult)
            V.tensor_scalar(sc2[:], sc2[:], 1.0, None, AL.is_ge)
            V.tensor_tensor(sc2[:], sc2[:], mm_xv[:], AL.mult)       # fix
            GP.tensor_tensor(x3(sc1[:]), bc(bvs[:]), x3(zt_xv[:]), AL.subtract)
            V.tensor_tensor(sc1[:], sc2[:], sc1[:], AL.mult)
            V.tensor_tensor(zt_xv[:], zt_xv[:], sc1[:], AL.add)
            GP.tensor_tensor(mm_xv[:], mm_xv[:], sc2[:], AL.subtract)
            calc_num_den()
        calc_lam()
        GP.tensor_scalar(t1s[:], lams[:], -1.0, LAMCAP, AL.mult, AL.add)
        V.tensor_tensor(t1s[:], t1s[:], infs[:], AL.mult)
        V.tensor_tensor(lams[:], lams[:], t1s[:], AL.add)
        V.tensor_tensor(x3(sc1[:]), bc(lams[:]), x3(g_xv[:]), AL.mult)
        V.tensor_tensor(sc1[:], sc1[:], p_xv[:], AL.subtract)
        V.tensor_scalar(sc1[:], sc1[:], 1.0, -1.0, AL.min, AL.max)
        for t in range(NT):
            nc.sync.dma_start(
                u_d[t * TILE:(t + 1) * TILE, :].rearrange(
                    "(b r s) j -> r b s j", b=8, r=128, s=2),
                sc1[:, 128 * t:128 * t + 128].rearrange(
                    "p (b s j) -> p b s j", b=8, s=2, j=8))


def _build():
    from concourse import bacc, mybir
    from concourse import tile as tile_mod
    from concourse._compat import axon_active
    f32 = mybir.dt.float32
    nc = bacc.Bacc("TRN2", target_bir_lowering=False,
                   debug=not axon_active(), num_devices=NCORES)
    x_d = nc.dram_tensor("x", [S, N], f32, kind="ExternalInput").ap()
    u_d = nc.dram_tensor("u", [S, N], f32, kind="ExternalOutput").ap()
    cds = {k: nc.dram_tensor(k, list(v), f32, kind="ExternalInput").ap()
           for k, v in _CSHAPES.items()}
    with tile_mod.TileContext(nc) as tc:
        build_kernel(nc, tc, x_d, u_d, cds)
    nc.compile()
    return nc


def kernel(x, W1, b1, W21, b21, W22, b22, W31, b31, W32, b32, A, G, mean, std):
    from concourse.bass_utils import run_bass_kernel_spmd
    f32 = np.float32
    x = np.asarray(x, f32)
    x0 = (x * np.asarray(std, f32) + np.asarray(mean, f32)).astype(f32)

    consts = _consts(np.asarray(W1, f32), np.asarray(b1, f32), np.asarray(W21, f32),
                     np.asarray(b21, f32), np.asarray(W22, f32), np.asarray(b22, f32),
                     np.asarray(W31, f32), np.asarray(b31, f32), np.asarray(W32, f32),
                     np.asarray(b32, f32), np.asarray(A, f32), np.asarray(G, f32))
    if "nc" not in _CACHE:
        _CACHE["nc"] = _build()
    nc = _CACHE["nc"]

    in_maps = []
    for c in range(NCORES):
        m = {"x": np.ascontiguousarray(x0[c * S:(c + 1) * S])}
        m.update(consts)
        in_maps.append(m)
    res = run_bass_kernel_spmd(nc, in_maps, list(range(NCORES)))
    out = np.concatenate([np.asarray(res.results[c]["u"]) for c in range(NCORES)],
                         axis=0)
    return out.astype(f32)



# revision 32
# speedup vs baseline: 2.0394x; 2.0394x over previous
"""Trainium2 Bass kernel for nn_BarrierPolicy (CBF-QP safety filter).

Data-parallel over batch: 8 cores x 32768 samples, f16 compute.

Phase A (per 4096-sample supertile): cast x to f16, DMA-transpose to SP2
layout, run the 3-layer MLP + dynamics matmuls on the tensor engine in f16
(1 cycle/row), evacuate with the scalar engine, DMA-transpose results back
to sample-major ("xview") layout.

Phase B (per 16384-sample chunk): Kiwiel variable-fixing active-set solve of
the per-sample box-QP dual in g-scaled space (no sign transform needed:
c(lam) = c0 + sum_j clip(lam*g_j^2 - p_j*g_j, -|g_j|, |g_j|)), T iterations
+ closed-form finish, then u = clip(-p + lam*g).  fc tensors are f16 (DVE
2x/4x modes), per-sample scalars f32.  Reduces and one broadcast-mult per
iteration run on the Pool engine to balance against DVE.

Layouts per 128-col tile block (2048 samples):
  xview: SBUF (128, 128): partition r, col 16b+8s0+j <-> sample 256b+2r+s0,
         coord j
  SP2  : block transpose of xview: partition 16b+8s0+j, col r
  slot : per-sample scalars (128, 16): partition r, col 2b+s0
"""
import numpy as np

B_FULL, N = 262144, 8
NCORES = 8
S = B_FULL // NCORES          # 32768 samples per core
NT = 16                       # 128-col xview tile blocks per core
NST = 8                       # supertiles (2 blocks each)
NCHUNK = 4                    # phase-B chunks (2 supertiles each)
CB = NT // NCHUNK             # tile blocks per chunk (4)
FC = CB * 128                 # fc cols per chunk (512)
SL = FC // 8                  # slot cols per chunk (64)
T_KIWIEL = 4
LAMCAP = float(2.0 ** 40)
LAM16CAP = 60000.0
EPS = 1e-12

_CACHE = {}

_CSHAPES = {
    "CF16": ((128, 1426), "f16"),   # TL1E | TL2W | TL3W | TDGW | TDAW
    "CF32": ((128, 3), "f32"),      # B1v | B2v | B3e
}


def _consts(W1, b1, W21, b21, W22, b22, W31, b31, W32, b32, A, G):
    f32, f16 = np.float32, np.float16
    TL1E = np.zeros((128, 8, 128), f32)
    for b in range(8):
        for s0 in range(2):
            TL1E[16 * b + 8 * s0:16 * b + 8 * s0 + 8, b,
                 64 * s0:64 * s0 + 64] = W1
    TL2W = np.zeros((128, 128), f32)
    for s0 in range(2):
        TL2W[64 * s0:64 * s0 + 64, 32 * s0:32 * s0 + 32] = W21
        TL2W[64 * s0:64 * s0 + 64, 64 + 32 * s0:64 + 32 * s0 + 32] = W22
    TL3W = np.zeros((128, 18), f32)
    for s0 in range(2):
        TL3W[32 * s0:32 * s0 + 32, 8 * s0:8 * s0 + 8] = W31
        TL3W[64 + 32 * s0:64 + 32 * s0 + 32, 16 + s0] = W32[:, 0]
    TDGW = np.kron(np.eye(16, dtype=f32), (-2.0 * G))
    TDAW = np.kron(np.eye(16, dtype=f32), A.T.astype(f32))
    CF16 = np.concatenate(
        [TL1E.reshape(128, 1024), TL2W, TL3W, TDGW, TDAW],
        axis=1).astype(f16)
    B1v = np.concatenate([b1, b1]).reshape(128, 1)
    B2v = np.concatenate([b21, b21, b22, b22]).reshape(128, 1)
    B3e = np.zeros((128, 1), f32)
    for h in range(2):
        for s0 in range(2):
            B3e[64 * h + 8 * s0:64 * h + 8 * s0 + 8, 0] = b31
            B3e[64 * h + 16 + s0, 0] = b32[0]
    CF32 = np.concatenate([B1v, B2v, B3e], axis=1).astype(f32)
    return {"CF16": CF16, "CF32": CF32}


def build_kernel(nc, tc, x_d, u_d, cds):
    from concourse import mybir
    f32 = mybir.dt.float32
    f16 = mybir.dt.float16
    AL = mybir.AluOpType
    AF = mybir.ActivationFunctionType
    XL = mybir.AxisListType.X
    V, GP, SC, PE, IO = nc.vector, nc.gpsimd, nc.scalar, nc.tensor, nc.sync

    x3 = lambda ap: ap.rearrange("p (c j) -> p c j", j=8)
    bc = lambda ap: ap.broadcast_to((128, SL, 8))

    with (
        tc.tile_pool(name="const", bufs=1) as cpool,
        tc.tile_pool(name="pers", bufs=1) as pers,
        tc.tile_pool(name="mlpsb", bufs=2) as mlpsb,
        tc.tile_pool(name="psMLP", bufs=1, space="PSUM") as psMLP,
        tc.tile_pool(name="psL3", bufs=1, space="PSUM") as psL3,
        tc.tile_pool(name="psDyn", bufs=2, space="PSUM") as psDyn,
    ):
        CT = {}
        for k, (shp, dt) in _CSHAPES.items():
            CT[k] = cpool.tile(list(shp), f16 if dt == "f16" else f32,
                               tag=k, name=k)
            SC.dma_start(CT[k][:], cds[k][:])
        C = {
            "TL1E": CT["CF16"][:, 0:1024].rearrange(
                "p (b r) -> p b r", b=8),
            "TL2W": CT["CF16"][:, 1024:1152],
            "TL3W": CT["CF16"][:, 1152:1170],
            "TDGW": CT["CF16"][:, 1170:1298],
            "TDAW": CT["CF16"][:, 1298:1426],
            "B1v": CT["CF32"][:, 0:1],
            "B2v": CT["CF32"][:, 1:2],
            "B3e": CT["CF32"][:, 2:3],
        }

        def fcf32(tag):
            return pers.tile([128, FC], f32, tag=tag, name=tag)

        def fcf16(tag):
            return pers.tile([128, FC], f16, tag=tag, name=tag)

        def slf32(tag):
            return pers.tile([128, SL], f32, tag=tag, name=tag)

        def slf16(tag):
            return pers.tile([128, SL], f16, tag=tag, name=tag)

        ch = []
        for c in range(NCHUNK):
            d = {}
            d["xv"] = fcf32(f"xv{c}")
            d["x16"] = fcf16(f"x16_{c}")
            d["xsp2"] = pers.tile([128, CB, 128], f16, tag=f"xsp2{c}",
                                  name=f"xsp2{c}")
            d["stg"] = pers.tile([128, 6 * CB, 128], f16, tag=f"stg{c}",
                                 name=f"stg{c}")
            d["stgT"] = pers.tile([128, 6 * CB, 128], f16, tag=f"stgT{c}",
                                  name=f"stgT{c}")
            for t in ["p16", "g16", "q", "pg", "gt", "ngt", "zt", "mm",
                      "z", "zr", "s", "w", "fx", "sq16"]:
                d[t] = fcf16(f"{t}_{c}")
            d["u32"] = fcf32(f"u32_{c}")
            for t in ["alog", "sxx", "lfr", "c0", "negc0", "viol", "infs",
                      "num", "den", "rE", "rK", "rM", "t1", "t2", "rc",
                      "lam", "nus", "bvs"]:
                d[t] = slf32(f"{t}_{c}")
            d["lam16"] = slf16(f"lam16_{c}")
            d["bvs16"] = slf16(f"bvs16_{c}")
            ch.append(d)

        # ---------------- Phase A ----------------
        def chunk_load(c):
            d = ch[c]
            IO.dma_start(
                d["xv"][:].rearrange("p (tb b s j) -> p tb b s j",
                                     tb=CB, b=8, s=2, j=8),
                x_d[c * CB * 2048:(c + 1) * CB * 2048, :].rearrange(
                    "(tb b r s) j -> r tb b s j", tb=CB, b=8, r=128, s=2))
            SC.activation(d["x16"][:], d["xv"][:], AF.Copy)
            IO.dma_start_transpose(d["xsp2"][:], d["x16"][:])

        def phase_a_supertile(st):
            c, l = st // 2, st % 2
            d = ch[c]
            xst = d["xsp2"][:, 2 * l:2 * l + 2, :].rearrange(
                "p a b -> p (a b)")
            h1P = psMLP.tile([128, 8, 256], f32, tag="mlpP",
                             name=f"h1P{st}")
            for b in range(8):
                PE.matmul(h1P[:, b, :], C["TL1E"][:, b, :], xst)
            h1 = mlpsb.tile([128, 8, 256], f16, tag="h1sb", name=f"h1_{st}")
            SC.activation(h1[:], h1P[:], AF.Relu, bias=C["B1v"])
            x2P = psMLP.tile([128, 8, 256], f32, tag="mlpP",
                             name=f"x2P{st}")
            for b in range(8):
                PE.matmul(x2P[:, b, :], C["TL2W"], h1[:, b, :])
            x2 = mlpsb.tile([128, 8, 256], f16, tag="x2sb", name=f"x2_{st}")
            SC.activation(x2[:], x2P[:], AF.Relu, bias=C["B2v"])
            l3P = psL3.tile([128, 4, 256], f32, tag="l3P", name=f"l3P{st}")
            for b in range(8):
                h, k = b % 2, b // 2
                PE.matmul(l3P[64 * h:64 * h + 18, k, :], C["TL3W"],
                          x2[:, b, :])
            SC.activation(d["stg"][:, 12 * l:12 * l + 8, :].rearrange(
                "p a b -> p (a b)"),
                l3P[:].rearrange("p a b -> p (a b)"),
                AF.Identity, bias=C["B3e"])
            dynP = psDyn.tile([128, 2, 256], f32, tag="dynP", name=f"dyn{st}")
            PE.matmul(dynP[:, 0, :], C["TDGW"], xst)
            PE.matmul(dynP[:, 1, :], C["TDAW"], xst)
            SC.activation(d["stg"][:, 12 * l + 8:12 * l + 12, :].rearrange(
                "p a b -> p (a b)"),
                dynP[:].rearrange("p a b -> p (a b)"), AF.Copy)

        def bwd_transpose(c):
            d = ch[c]
            SC.dma_start_transpose(
                d["stgT"][:], d["stg"][:].rearrange("p a b -> p (a b)"))

        def extracts(c):
            d = ch[c]
            srcall = d["stgT"][:].rearrange(
                "p (B two) (h gg s j) -> p B two h gg s j",
                B=12, two=2, h=2, gg=4, s=2, j=8)
            pview = d["p16"][:].rearrange(
                "p (tb k h s j) -> p tb k h s j", tb=CB, k=4, h=2, s=2, j=8)
            aview = d["alog"][:].rearrange(
                "p (tb k h s) -> p tb k h s", tb=CB, k=4, h=2, s=2)
            blkview = d["stgT"][:]
            for l in range(2):
                for i in range(2):
                    V.tensor_copy(pview[:, 2 * l + i, :, :, :, :],
                                  srcall[:, 6 * l:6 * l + 4, i, :, 0, :, :])
                    V.tensor_copy(aview[:, 2 * l + i, :, :, :],
                                  srcall[:, 6 * l:6 * l + 4, i, :, 1, 0, 0:2])
                V.tensor_copy(
                    d["g16"][:, 256 * l:256 * l + 256],
                    blkview[:, 12 * l + 8:12 * l + 10, :].rearrange(
                        "p a b -> p (a b)"))
                V.tensor_copy(
                    d["sq16"][:, 256 * l:256 * l + 256],
                    blkview[:, 12 * l + 10:12 * l + 12, :].rearrange(
                        "p a b -> p (a b)"))

        # ---------------- Phase B ----------------
        def calc_lam(d):
            V.tensor_scalar(d["t2"][:], d["den"][:], EPS, None, AL.add)
            V.reciprocal(d["rc"][:], d["t2"][:])
            V.scalar_tensor_tensor(d["lam"][:], d["num"][:], -1.0, d["rc"][:],
                                   AL.mult, AL.mult)
            V.tensor_tensor(d["lam"][:], d["lam"][:], d["viol"][:], AL.mult)
            V.tensor_scalar(d["lam16"][:], d["lam"][:], LAM16CAP, None,
                            AL.min)

        def phase_b_setup(c):
            d = ch[c]
            GP.tensor_tensor(d["q"][:], d["g16"][:], d["g16"][:], AL.mult)
            V.tensor_tensor(d["pg"][:], d["p16"][:], d["g16"][:], AL.mult)
            SC.activation(d["gt"][:], d["g16"][:], AF.Abs)
            V.tensor_scalar(d["ngt"][:], d["gt"][:], -1.0, None, AL.mult)
            GP.tensor_scalar(d["zt"][:], d["pg"][:], -1.0, None, AL.mult)
            GP.memset(d["mm"][:], 1.0)
            # c(0) partials: sum clip(pg, +-gt) = -sum s0
            V.tensor_tensor(d["s"][:], d["pg"][:], d["gt"][:], AL.min)
            V.tensor_tensor(d["s"][:], d["s"][:], d["ngt"][:], AL.max)
            V.tensor_reduce(d["rE"][:], x3(d["s"][:]), XL, AL.add)
            V.tensor_reduce(d["rK"][:], x3(d["zt"][:]), XL, AL.add)
            V.tensor_reduce(d["den"][:], x3(d["q"][:]), XL, AL.add)
            # c0 = -2*sum(x*ax) + 4*sigmoid(alog)*(16 - sxx)
            V.tensor_tensor(d["w"][:], d["sq16"][:], d["x16"][:], AL.mult)
            V.tensor_reduce(d["lfr"][:], x3(d["w"][:]), XL, AL.add)
            SC.activation(d["z"][:], d["x16"][:], AF.Square)
            V.tensor_reduce(d["sxx"][:], x3(d["z"][:]), XL, AL.add)
            SC.activation(d["nus"][:], d["alog"][:], AF.Sigmoid)
            V.tensor_scalar(d["t2"][:], d["sxx"][:], -4.0, 64.0, AL.mult,
                            AL.add)
            V.tensor_tensor(d["c0"][:], d["nus"][:], d["t2"][:], AL.mult)
            V.scalar_tensor_tensor(d["c0"][:], d["lfr"][:], -2.0, d["c0"][:],
                                   AL.mult, AL.add)
            V.tensor_scalar(d["negc0"][:], d["c0"][:], -1.0, None, AL.mult)
            V.tensor_tensor(d["viol"][:], d["rE"][:], d["c0"][:], AL.is_gt)
            V.tensor_tensor(d["num"][:], d["rK"][:], d["c0"][:], AL.add)

        def phase_b_iter(c):
            d = ch[c]
            calc_lam(d)
            V.tensor_tensor(x3(d["z"][:]), bc(d["lam16"][:]),
                            x3(d["q"][:]), AL.mult)
            V.tensor_tensor(d["zr"][:], d["z"][:], d["pg"][:], AL.subtract)
            V.tensor_tensor(d["s"][:], d["zr"][:], d["gt"][:], AL.min)
            V.tensor_tensor(d["s"][:], d["s"][:], d["ngt"][:], AL.max)
            V.tensor_reduce(d["rE"][:], x3(d["s"][:]), XL, AL.add)
            V.tensor_tensor(d["nus"][:], d["rE"][:], d["negc0"][:], AL.is_lt)
            V.tensor_scalar(d["bvs16"][:], d["nus"][:], 2.0, -1.0, AL.mult,
                            AL.add)
            GP.tensor_tensor(x3(d["w"][:]), bc(d["bvs16"][:]),
                             x3(d["zr"][:]), AL.mult)
            V.tensor_tensor(d["fx"][:], d["w"][:], d["gt"][:], AL.is_ge)
            GP.tensor_tensor(d["fx"][:], d["fx"][:], d["mm"][:], AL.mult)
            V.tensor_tensor(d["s"][:], d["s"][:], d["zt"][:], AL.subtract)
            V.tensor_tensor(d["s"][:], d["s"][:], d["fx"][:], AL.mult)
            V.tensor_tensor(d["zt"][:], d["zt"][:], d["s"][:], AL.add)
            GP.tensor_tensor(d["mm"][:], d["mm"][:], d["fx"][:], AL.subtract)
            GP.tensor_tensor(d["w"][:], d["q"][:], d["mm"][:], AL.mult)
            V.tensor_reduce(d["rK"][:], x3(d["zt"][:]), XL, AL.add)
            V.tensor_reduce(d["den"][:], x3(d["w"][:]), XL, AL.add)
            V.tensor_tensor(d["num"][:], d["rK"][:], d["c0"][:], AL.add)

        def phase_b_final(c):
            d = ch[c]
            calc_lam(d)
            # infeasible detection: den -> 0 with num still negative
            V.tensor_scalar(d["t1"][:], d["den"][:], 1e-4, None, AL.is_lt)
            V.tensor_scalar(d["t2"][:], d["num"][:], 0.0, None, AL.is_lt)
            V.tensor_tensor(d["infs"][:], d["t1"][:], d["t2"][:], AL.mult)
            V.tensor_tensor(d["infs"][:], d["infs"][:], d["viol"][:], AL.mult)
            V.tensor_scalar(d["t1"][:], d["lam"][:], -1.0, LAMCAP, AL.mult,
                            AL.add)
            V.tensor_tensor(d["t1"][:], d["t1"][:], d["infs"][:], AL.mult)
            V.tensor_tensor(d["lam"][:], d["lam"][:], d["t1"][:], AL.add)
            V.tensor_scalar(d["lam16"][:], d["lam"][:], LAM16CAP, None,
                            AL.min)
            V.tensor_tensor(x3(d["z"][:]), bc(d["lam16"][:]),
                            x3(d["g16"][:]), AL.mult)
            V.tensor_tensor(d["z"][:], d["z"][:], d["p16"][:], AL.subtract)
            V.tensor_scalar(d["u32"][:], d["z"][:], 1.0, -1.0, AL.min, AL.max)
            SC.dma_start(
                u_d[c * CB * 2048:(c + 1) * CB * 2048, :].rearrange(
                    "(tb b r s) j -> r tb b s j", tb=CB, b=8, r=128, s=2),
                d["u32"][:].rearrange("p (tb b s j) -> p tb b s j",
                                      tb=CB, b=8, s=2, j=8))

        # ---------------- emission ----------------
        # skewed software pipeline: chunk c's B-solve interleaves with later
        # chunks' A-compute and earlier chunks' remaining iterations
        left = [T_KIWIEL] * NCHUNK
        started = []

        def pump():
            for cc in reversed(started):
                if left[cc] > 0:
                    phase_b_iter(cc)
                    left[cc] -= 1
                    if left[cc] == 0:
                        phase_b_final(cc)

        for c in range(NCHUNK):
            chunk_load(c)
        for c in range(NCHUNK):
            phase_a_supertile(2 * c)
            phase_a_supertile(2 * c + 1)
            bwd_transpose(c)
            if c >= 1:
                prev = c - 1
                extracts(prev)
                phase_b_setup(prev)
                started.append(prev)
                pump()
        extracts(NCHUNK - 1)
        phase_b_setup(NCHUNK - 1)
        started.append(NCHUNK - 1)
        while any(left):
            pump()


def _build():
    from concourse import bacc, mybir
    from concourse import tile as tile_mod
    from concourse._compat import axon_active
    f32 = mybir.dt.float32
    f16 = mybir.dt.float16
    nc = bacc.Bacc("TRN2", target_bir_lowering=False,
                   debug=not axon_active(), num_devices=NCORES)
    x_d = nc.dram_tensor("x", [S, N], f32, kind="ExternalInput").ap()
    u_d = nc.dram_tensor("u", [S, N], f32, kind="ExternalOutput").ap()
    cds = {}
    for k, (shp, dt) in _CSHAPES.items():
        cds[k] = nc.dram_tensor(k, list(shp), f16 if dt == "f16" else f32,
                                kind="ExternalInput").ap()
    with tile_mod.TileContext(nc) as tc:
        build_kernel(nc, tc, x_d, u_d, cds)
    nc.compile()
    return nc


def kernel(x, W1, b1, W21, b21, W22, b22, W31, b31, W32, b32, A, G, mean, std):
    from concourse.bass_utils import run_bass_kernel_spmd
    f32 = np.float32
    x = np.asarray(x, f32)
    x0 = (x * np.asarray(std, f32) + np.asarray(mean, f32)).astype(f32)

    consts = _consts(np.asarray(W1, f32), np.asarray(b1, f32),
                     np.asarray(W21, f32), np.asarray(b21, f32),
                     np.asarray(W22, f32), np.asarray(b22, f32),
                     np.asarray(W31, f32), np.asarray(b31, f32),
                     np.asarray(W32, f32), np.asarray(b32, f32),
                     np.asarray(A, f32), np.asarray(G, f32))
    if "nc" not in _CACHE:
        _CACHE["nc"] = _build()
    nc = _CACHE["nc"]

    in_maps = []
    for c in range(NCORES):
        m = {"x": np.ascontiguousarray(x0[c * S:(c + 1) * S])}
        m.update(consts)
        in_maps.append(m)
    res = run_bass_kernel_spmd(nc, in_maps, list(range(NCORES)))
    out = np.concatenate([np.asarray(res.results[c]["u"])
                          for c in range(NCORES)], axis=0)
    return out.astype(f32)


# revision 44
# speedup vs baseline: 2.3110x; 1.1332x over previous
"""Trainium2 Bass kernel for nn_BarrierPolicy (CBF-QP safety filter).

Data-parallel over batch: 8 cores x 32768 samples, f16 compute.

Phase A (per 4096-sample supertile): cast x to f16, DMA-transpose to SP2
layout, run the 3-layer MLP + dynamics matmuls on the tensor engine in f16
(1 cycle/row), evacuate with the scalar engine, DMA-transpose results back
to sample-major ("xview") layout.

Phase B (per 16384-sample chunk): Kiwiel variable-fixing active-set solve of
the per-sample box-QP dual in g-scaled space (no sign transform needed:
c(lam) = c0 + sum_j clip(lam*g_j^2 - p_j*g_j, -|g_j|, |g_j|)), T iterations
+ closed-form finish, then u = clip(-p + lam*g).  fc tensors are f16 (DVE
2x/4x modes), per-sample scalars f32.  Reduces and one broadcast-mult per
iteration run on the Pool engine to balance against DVE.

Layouts per 128-col tile block (2048 samples):
  xview: SBUF (128, 128): partition r, col 16b+8s0+j <-> sample 256b+2r+s0,
         coord j
  SP2  : block transpose of xview: partition 16b+8s0+j, col r
  slot : per-sample scalars (128, 16): partition r, col 2b+s0
"""
import numpy as np

B_FULL, N = 262144, 8
NCORES = 8
S = B_FULL // NCORES          # 32768 samples per core
NT = 16                       # 128-col xview tile blocks per core
NST = 8                       # supertiles (2 blocks each)
NCHUNK = 4                    # phase-B chunks (2 supertiles each)
CB = NT // NCHUNK             # tile blocks per chunk (4)
FC = CB * 128                 # fc cols per chunk (512)
SL = FC // 8                  # slot cols per chunk (64)
T_KIWIEL = 3
LAMCAP = float(2.0 ** 40)
LAM16CAP = 60000.0
EPS = 1e-12

_CACHE = {}

_CSHAPES = {
    "CF16": ((128, 1426), "f16"),   # TL1E | TL2W | TL3W | TDGW | TDAW
    "CF32": ((128, 3), "f32"),      # B1v | B2v | B3e
}


def _consts(W1, b1, W21, b21, W22, b22, W31, b31, W32, b32, A, G):
    f32, f16 = np.float32, np.float16
    TL1E = np.zeros((128, 8, 128), f32)
    for b in range(8):
        for s0 in range(2):
            TL1E[16 * b + 8 * s0:16 * b + 8 * s0 + 8, b,
                 64 * s0:64 * s0 + 64] = W1
    TL2W = np.zeros((128, 128), f32)
    for s0 in range(2):
        TL2W[64 * s0:64 * s0 + 64, 32 * s0:32 * s0 + 32] = W21
        TL2W[64 * s0:64 * s0 + 64, 64 + 32 * s0:64 + 32 * s0 + 32] = W22
    TL3W = np.zeros((128, 18), f32)
    for s0 in range(2):
        TL3W[32 * s0:32 * s0 + 32, 8 * s0:8 * s0 + 8] = W31
        TL3W[64 + 32 * s0:64 + 32 * s0 + 32, 16 + s0] = W32[:, 0]
    TDGW = np.kron(np.eye(16, dtype=f32), (-2.0 * G))
    TDAW = np.kron(np.eye(16, dtype=f32), A.T.astype(f32))
    CF16 = np.concatenate(
        [TL1E.reshape(128, 1024), TL2W, TL3W, TDGW, TDAW],
        axis=1).astype(f16)
    B1v = np.concatenate([b1, b1]).reshape(128, 1)
    B2v = np.concatenate([b21, b21, b22, b22]).reshape(128, 1)
    B3e = np.zeros((128, 1), f32)
    for h in range(2):
        for s0 in range(2):
            B3e[64 * h + 8 * s0:64 * h + 8 * s0 + 8, 0] = b31
            B3e[64 * h + 16 + s0, 0] = b32[0]
    CF32 = np.concatenate([B1v, B2v, B3e], axis=1).astype(f32)
    return {"CF16": CF16, "CF32": CF32}


def build_kernel(nc, tc, x_d, u_d, cds):
    from concourse import mybir
    f32 = mybir.dt.float32
    f16 = mybir.dt.float16
    AL = mybir.AluOpType
    AF = mybir.ActivationFunctionType
    XL = mybir.AxisListType.X
    V, GP, SC, PE, IO = nc.vector, nc.gpsimd, nc.scalar, nc.tensor, nc.sync

    x3 = lambda ap: ap.rearrange("p (c j) -> p c j", j=8)
    bc = lambda ap: ap.broadcast_to((128, SL, 8))

    with (
        tc.tile_pool(name="const", bufs=1) as cpool,
        tc.tile_pool(name="pers", bufs=1) as pers,
        tc.tile_pool(name="mlpsb", bufs=2) as mlpsb,
        tc.tile_pool(name="psMLP", bufs=1, space="PSUM") as psMLP,
        tc.tile_pool(name="psL3", bufs=1, space="PSUM") as psL3,
        tc.tile_pool(name="psDyn", bufs=2, space="PSUM") as psDyn,
    ):
        CT = {}
        for k, (shp, dt) in _CSHAPES.items():
            CT[k] = cpool.tile(list(shp), f16 if dt == "f16" else f32,
                               tag=k, name=k)

        def load_consts():
            for k in _CSHAPES:
                IO.dma_start(CT[k][:], cds[k][:])
        C = {
            "TL1E": CT["CF16"][:, 0:1024].rearrange(
                "p (b r) -> p b r", b=8),
            "TL2W": CT["CF16"][:, 1024:1152],
            "TL3W": CT["CF16"][:, 1152:1170],
            "TDGW": CT["CF16"][:, 1170:1298],
            "TDAW": CT["CF16"][:, 1298:1426],
            "B1v": CT["CF32"][:, 0:1],
            "B2v": CT["CF32"][:, 1:2],
            "B3e": CT["CF32"][:, 2:3],
        }

        def fcf32(tag):
            return pers.tile([128, FC], f32, tag=tag, name=tag)

        def fcf16(tag):
            return pers.tile([128, FC], f16, tag=tag, name=tag)

        def slf32(tag):
            return pers.tile([128, SL], f32, tag=tag, name=tag)

        def slf16(tag):
            return pers.tile([128, SL], f16, tag=tag, name=tag)

        ch = []
        for c in range(NCHUNK):
            d = {}
            d["xv"] = fcf32(f"xv{c}")
            d["x16"] = fcf16(f"x16_{c}")
            d["xsp2"] = pers.tile([128, CB, 128], f16, tag=f"xsp2{c}",
                                  name=f"xsp2{c}")
            d["stg"] = pers.tile([128, 6 * CB, 128], f16, tag=f"stg{c}",
                                 name=f"stg{c}")
            d["stgT"] = pers.tile([128, 6 * CB, 128], f16, tag=f"stgT{c}",
                                  name=f"stgT{c}")
            for t in ["p16", "g16", "q", "pg", "gt", "ngt", "zt", "mm",
                      "z", "zr", "s", "w", "fx", "sq16"]:
                d[t] = fcf16(f"{t}_{c}")
            d["u32"] = fcf32(f"u32_{c}")
            for t in ["alog", "sxx", "lfr", "c0", "negc0", "viol", "infs",
                      "num", "den", "rE", "rK", "rM", "t1", "t2", "rc",
                      "lam", "nus", "bvs"]:
                d[t] = slf32(f"{t}_{c}")
            d["lam16"] = slf16(f"lam16_{c}")
            d["bvs16"] = slf16(f"bvs16_{c}")
            ch.append(d)

        # ---------------- Phase A ----------------
        def chunk_load(c):
            d = ch[c]
            IO.dma_start(
                d["xv"][:].rearrange("p (tb b s j) -> p tb b s j",
                                     tb=CB, b=8, s=2, j=8),
                x_d[c * CB * 2048:(c + 1) * CB * 2048, :].rearrange(
                    "(tb b r s) j -> r tb b s j", tb=CB, b=8, r=128, s=2))
            SC.activation(d["x16"][:], d["xv"][:], AF.Copy)
            SC.dma_start_transpose(d["xsp2"][:], d["x16"][:])

        def phase_a_supertile(st):
            c, l = st // 2, st % 2
            d = ch[c]
            xst = d["xsp2"][:, 2 * l:2 * l + 2, :].rearrange(
                "p a b -> p (a b)")
            l3P = psL3.tile([128, 4, 256], f32, tag="l3P", name=f"l3P{st}")
            for hf in range(2):
                h1P = psMLP.tile([128, 4, 256], f32, tag="mlpP",
                                 name=f"h1P{st}{hf}")
                for bi in range(4):
                    PE.matmul(h1P[:, bi, :], C["TL1E"][:, 4 * hf + bi, :],
                              xst)
                h1 = mlpsb.tile([128, 4, 256], f16, tag="h1sb",
                                name=f"h1_{st}{hf}")
                SC.activation(h1[:], h1P[:], AF.Relu, bias=C["B1v"])
                x2P = psMLP.tile([128, 4, 256], f32, tag="mlpP",
                                 name=f"x2P{st}{hf}")
                for bi in range(4):
                    PE.matmul(x2P[:, bi, :], C["TL2W"], h1[:, bi, :])
                x2 = mlpsb.tile([128, 4, 256], f16, tag="x2sb",
                                name=f"x2_{st}{hf}")
                SC.activation(x2[:], x2P[:], AF.Relu, bias=C["B2v"])
                for bi in range(4):
                    b = 4 * hf + bi
                    h, k = b % 2, b // 2
                    PE.matmul(l3P[64 * h:64 * h + 18, k, :], C["TL3W"],
                              x2[:, bi, :])
            SC.activation(d["stg"][:, 12 * l:12 * l + 8, :].rearrange(
                "p a b -> p (a b)"),
                l3P[:].rearrange("p a b -> p (a b)"),
                AF.Identity, bias=C["B3e"])
            dynP = psDyn.tile([128, 2, 256], f32, tag="dynP", name=f"dyn{st}")
            PE.matmul(dynP[:, 0, :], C["TDGW"], xst)
            PE.matmul(dynP[:, 1, :], C["TDAW"], xst)
            SC.activation(d["stg"][:, 12 * l + 8:12 * l + 12, :].rearrange(
                "p a b -> p (a b)"),
                dynP[:].rearrange("p a b -> p (a b)"), AF.Copy)

        def bwd_transpose(c):
            d = ch[c]
            IO.dma_start_transpose(
                d["stgT"][:], d["stg"][:].rearrange("p a b -> p (a b)"))

        def extracts(c):
            d = ch[c]
            srcall = d["stgT"][:].rearrange(
                "p (B two) (h gg s j) -> p B two h gg s j",
                B=12, two=2, h=2, gg=4, s=2, j=8)
            pview = d["p16"][:].rearrange(
                "p (tb k h s j) -> p tb k h s j", tb=CB, k=4, h=2, s=2, j=8)
            aview = d["alog"][:].rearrange(
                "p (tb k h s) -> p tb k h s", tb=CB, k=4, h=2, s=2)
            blkview = d["stgT"][:]
            for l in range(2):
                for i in range(2):
                    V.tensor_copy(pview[:, 2 * l + i, :, :, :, :],
                                  srcall[:, 6 * l:6 * l + 4, i, :, 0, :, :])
                    V.tensor_copy(aview[:, 2 * l + i, :, :, :],
                                  srcall[:, 6 * l:6 * l + 4, i, :, 1, 0, 0:2])
                V.tensor_copy(
                    d["g16"][:, 256 * l:256 * l + 256],
                    blkview[:, 12 * l + 8:12 * l + 10, :].rearrange(
                        "p a b -> p (a b)"))
                V.tensor_copy(
                    d["sq16"][:, 256 * l:256 * l + 256],
                    blkview[:, 12 * l + 10:12 * l + 12, :].rearrange(
                        "p a b -> p (a b)"))

        # ---------------- Phase B ----------------
        def calc_lam(d):
            V.tensor_scalar(d["t2"][:], d["den"][:], EPS, None, AL.add)
            V.reciprocal(d["rc"][:], d["t2"][:])
            V.scalar_tensor_tensor(d["lam"][:], d["num"][:], -1.0, d["rc"][:],
                                   AL.mult, AL.mult)
            V.tensor_tensor(d["lam"][:], d["lam"][:], d["viol"][:], AL.mult)
            V.tensor_scalar(d["lam16"][:], d["lam"][:], LAM16CAP, None,
                            AL.min)

        def phase_b_setup(c):
            d = ch[c]
            u16 = mybir.dt.uint16
            GP.tensor_tensor(d["q"][:], d["g16"][:], d["g16"][:], AL.mult)
            V.tensor_tensor(d["pg"][:], d["p16"][:], d["g16"][:], AL.mult)
            V.tensor_scalar(d["gt"][:].bitcast(u16), d["g16"][:].bitcast(u16),
                            0x7FFF, None, AL.bitwise_and)
            V.tensor_scalar(d["ngt"][:].bitcast(u16), d["gt"][:].bitcast(u16),
                            0x8000, None, AL.bitwise_or)
            GP.tensor_scalar(d["zt"][:], d["pg"][:], -1.0, None, AL.mult)
            GP.memset(d["mm"][:], 1.0)
            # c(0) partials: sum clip(pg, +-gt) = -sum s0
            V.tensor_tensor(d["s"][:], d["pg"][:], d["gt"][:], AL.min)
            V.tensor_tensor(d["s"][:], d["s"][:], d["ngt"][:], AL.max)
            V.tensor_reduce(d["rE"][:], x3(d["s"][:]), XL, AL.add)
            V.tensor_reduce(d["rK"][:], x3(d["zt"][:]), XL, AL.add)
            V.tensor_reduce(d["den"][:], x3(d["q"][:]), XL, AL.add)
            # c0 = -2*sum(x*ax) + 4*sigmoid(alog)*(16 - sxx)
            V.tensor_tensor(d["w"][:], d["sq16"][:], d["x16"][:], AL.mult)
            V.tensor_reduce(d["lfr"][:], x3(d["w"][:]), XL, AL.add)
            GP.tensor_tensor(d["z"][:], d["x16"][:], d["x16"][:], AL.mult)
            V.tensor_reduce(d["sxx"][:], x3(d["z"][:]), XL, AL.add)
            SC.activation(d["nus"][:], d["alog"][:], AF.Sigmoid)
            V.tensor_scalar(d["t2"][:], d["sxx"][:], -4.0, 64.0, AL.mult,
                            AL.add)
            V.tensor_tensor(d["c0"][:], d["nus"][:], d["t2"][:], AL.mult)
            V.scalar_tensor_tensor(d["c0"][:], d["lfr"][:], -2.0, d["c0"][:],
                                   AL.mult, AL.add)
            V.tensor_scalar(d["negc0"][:], d["c0"][:], -1.0, None, AL.mult)
            V.tensor_tensor(d["viol"][:], d["rE"][:], d["c0"][:], AL.is_gt)
            V.tensor_tensor(d["num"][:], d["rK"][:], d["c0"][:], AL.add)

        def phase_b_iter(c):
            d = ch[c]
            calc_lam(d)
            V.tensor_tensor(x3(d["z"][:]), bc(d["lam16"][:]),
                            x3(d["q"][:]), AL.mult)
            V.tensor_tensor(d["zr"][:], d["z"][:], d["pg"][:], AL.subtract)
            V.tensor_tensor(d["s"][:], d["zr"][:], d["gt"][:], AL.min)
            V.tensor_tensor(d["s"][:], d["s"][:], d["ngt"][:], AL.max)
            V.tensor_reduce(d["rE"][:], x3(d["s"][:]), XL, AL.add)
            V.tensor_tensor(d["nus"][:], d["rE"][:], d["negc0"][:], AL.is_lt)
            V.tensor_scalar(d["bvs16"][:], d["nus"][:], 2.0, -1.0, AL.mult,
                            AL.add)
            GP.tensor_tensor(x3(d["w"][:]), bc(d["bvs16"][:]),
                             x3(d["zr"][:]), AL.mult)
            V.tensor_tensor(d["fx"][:], d["w"][:], d["gt"][:], AL.is_ge)
            GP.tensor_tensor(d["fx"][:], d["fx"][:], d["mm"][:], AL.mult)
            V.tensor_tensor(d["s"][:], d["s"][:], d["zt"][:], AL.subtract)
            V.tensor_tensor(d["s"][:], d["s"][:], d["fx"][:], AL.mult)
            V.tensor_tensor(d["zt"][:], d["zt"][:], d["s"][:], AL.add)
            GP.tensor_tensor(d["mm"][:], d["mm"][:], d["fx"][:], AL.subtract)
            GP.tensor_tensor(d["w"][:], d["q"][:], d["mm"][:], AL.mult)
            V.tensor_reduce(d["rK"][:], x3(d["zt"][:]), XL, AL.add)
            V.tensor_reduce(d["den"][:], x3(d["w"][:]), XL, AL.add)
            V.tensor_tensor(d["num"][:], d["rK"][:], d["c0"][:], AL.add)

        def phase_b_final(c):
            d = ch[c]
            calc_lam(d)
            # infeasible detection: den -> 0 with num still negative
            V.tensor_scalar(d["t1"][:], d["den"][:], 1e-4, None, AL.is_lt)
            V.tensor_scalar(d["t2"][:], d["num"][:], 0.0, None, AL.is_lt)
            V.tensor_tensor(d["infs"][:], d["t1"][:], d["t2"][:], AL.mult)
            V.tensor_tensor(d["infs"][:], d["infs"][:], d["viol"][:], AL.mult)
            V.tensor_scalar(d["t1"][:], d["lam"][:], -1.0, LAMCAP, AL.mult,
                            AL.add)
            V.tensor_tensor(d["t1"][:], d["t1"][:], d["infs"][:], AL.mult)
            V.tensor_tensor(d["lam"][:], d["lam"][:], d["t1"][:], AL.add)
            V.tensor_scalar(d["lam16"][:], d["lam"][:], LAM16CAP, None,
                            AL.min)
            V.tensor_tensor(x3(d["z"][:]), bc(d["lam16"][:]),
                            x3(d["g16"][:]), AL.mult)
            V.tensor_tensor(d["z"][:], d["z"][:], d["p16"][:], AL.subtract)
            V.tensor_scalar(d["u32"][:], d["z"][:], 1.0, -1.0, AL.min, AL.max)
            IO.dma_start(
                u_d[c * CB * 2048:(c + 1) * CB * 2048, :].rearrange(
                    "(tb b r s) j -> r tb b s j", tb=CB, b=8, r=128, s=2),
                d["u32"][:].rearrange("p (tb b s j) -> p tb b s j",
                                      tb=CB, b=8, s=2, j=8))

        # ---------------- emission ----------------
        # skewed software pipeline: chunk c's B-solve interleaves with later
        # chunks' A-compute and earlier chunks' remaining iterations
        left = [T_KIWIEL] * NCHUNK
        started = []

        def pump():
            for cc in reversed(started):
                if left[cc] > 0:
                    phase_b_iter(cc)
                    left[cc] -= 1
                    if left[cc] == 0:
                        phase_b_final(cc)

        for c in range(NCHUNK):
            chunk_load(c)
            if c == 0:
                load_consts()
            phase_a_supertile(2 * c)
            phase_a_supertile(2 * c + 1)
            bwd_transpose(c)
            if c >= 1:
                prev = c - 1
                extracts(prev)
                phase_b_setup(prev)
                started.append(prev)
                pump()
        extracts(NCHUNK - 1)
        phase_b_setup(NCHUNK - 1)
        started.append(NCHUNK - 1)
        while any(left):
            pump()


def _build():
    from concourse import bacc, mybir
    from concourse import tile as tile_mod
    from concourse._compat import axon_active
    f32 = mybir.dt.float32
    f16 = mybir.dt.float16
    nc = bacc.Bacc("TRN2", target_bir_lowering=False,
                   debug=not axon_active(), num_devices=NCORES)
    x_d = nc.dram_tensor("x", [S, N], f32, kind="ExternalInput").ap()
    u_d = nc.dram_tensor("u", [S, N], f32, kind="ExternalOutput").ap()
    cds = {}
    for k, (shp, dt) in _CSHAPES.items():
        cds[k] = nc.dram_tensor(k, list(shp), f16 if dt == "f16" else f32,
                                kind="ExternalInput").ap()
    with tile_mod.TileContext(nc) as tc:
        build_kernel(nc, tc, x_d, u_d, cds)
    nc.compile()
    return nc


def kernel(x, W1, b1, W21, b21, W22, b22, W31, b31, W32, b32, A, G, mean, std):
    from concourse.bass_utils import run_bass_kernel_spmd
    f32 = np.float32
    x = np.asarray(x, f32)
    x0 = (x * np.asarray(std, f32) + np.asarray(mean, f32)).astype(f32)

    consts = _consts(np.asarray(W1, f32), np.asarray(b1, f32),
                     np.asarray(W21, f32), np.asarray(b21, f32),
                     np.asarray(W22, f32), np.asarray(b22, f32),
                     np.asarray(W31, f32), np.asarray(b31, f32),
                     np.asarray(W32, f32), np.asarray(b32, f32),
                     np.asarray(A, f32), np.asarray(G, f32))
    if "nc" not in _CACHE:
        _CACHE["nc"] = _build()
    nc = _CACHE["nc"]

    in_maps = []
    for c in range(NCORES):
        m = {"x": np.ascontiguousarray(x0[c * S:(c + 1) * S])}
        m.update(consts)
        in_maps.append(m)
    res = run_bass_kernel_spmd(nc, in_maps, list(range(NCORES)))
    out = np.concatenate([np.asarray(res.results[c]["u"])
                          for c in range(NCORES)], axis=0)
    return out.astype(f32)
